# revision 1
# baseline (speedup 1.0000x reference)
"""NeuralAdditiveModel TRN2 kernel.

out[b] = sum_f ( relu(relu(x[b,f]*W1[f,:]+b1[f,:]) @ W2[f] + b2[f]) @ W3[f] + b3[f] ) + bias

Sharding: data-parallel over batch, 8 cores x 1024 rows. No collectives.

Per-core dataflow (matmul operands bf16, PSUM/bias fp32):
  z1[f] = K=2 matmul([W1row; b1row], [xTrow; ones])  (4-way row-tiled, PE)
  h1[f] = relu(z1) drain PSUM->SBUF on ACT/DVE
  z2[f] = W2[f].T @ h1[f]  (col-tiled feature pairs, PE)
  h2[f] = relu(z2 + b2) drain on DVE/ACT
  out  += W3pair.T @ h2pair (M=1 accumulating matmuls, 2 col positions)
"""

import os
import sys
from contextlib import ExitStack

import numpy as np

sys.path.insert(0, "/opt/trn_rl_repo")

import concourse.bass as bass  # noqa: E402
import concourse.tile as tile  # noqa: E402
from concourse import bacc, mybir  # noqa: E402
from concourse.bass_utils import run_bass_kernel_spmd  # noqa: E402

B, F, S, H1 = 8192, 128, 128, 64
NCORES = 8
BLOC = B // NCORES  # 1024 rows per core
BT = 512            # batch chunk (PSUM bank width in fp32)
NBT = BLOC // BT    # 2
F32 = mybir.dt.float32
BF16 = mybir.dt.bfloat16

_CACHE = {}


def _build(variant="full", use_f32r=False, repeat=1):
    """Trace + compile the per-core Bass program (identical on all cores).

    variant: 'full' | 'pe_only' (no relu drains; mains read a constant tile)
             | 'drains_only' (no recurring matmuls)
    use_f32r: store activations/weights as fp32, feed matmuls as float32r.
    repeat: re-execute the whole body this many times (benchmark amplifier).
    """
    TD = F32 if use_f32r else BF16  # storage dtype of matmul operands

    def mv(ap):  # matmul-operand view
        return ap.bitcast(mybir.dt.float32r) if use_f32r else ap

    nc = bacc.Bacc(
        "TRN2",
        target_bir_lowering=False,
        debug=False,
        enable_asserts=False,
        num_devices=NCORES,
    )

    xTg_d = nc.dram_tensor("xTg", [4, 32, BLOC], TD, kind="ExternalInput").ap()
    w1q_d = nc.dram_tensor("w1q", [4, 32 * S], TD, kind="ExternalInput").ap()
    b1q_d = nc.dram_tensor("b1q", [4, 32 * S], TD, kind="ExternalInput").ap()
    w2t_d = nc.dram_tensor("w2t", [S, F * H1], TD, kind="ExternalInput").ap()
    b2p_d = nc.dram_tensor("b2p", [2 * H1, 2 * 32], F32, kind="ExternalInput").ap()
    w3p_d = nc.dram_tensor("w3p", [2 * H1, 2 * 32], TD, kind="ExternalInput").ap()
    ones_d = nc.dram_tensor("ones", [1, 32 * BT], TD, kind="ExternalInput").ap()
    out_d = nc.dram_tensor("out", [NBT * 2, BT], F32, kind="ExternalOutput").ap()

    Relu = mybir.ActivationFunctionType.Relu

    with tile.TileContext(nc) as tc, ExitStack() as ctx:
        singles = ctx.enter_context(tc.tile_pool(name="singles", bufs=1))
        h1_pool = ctx.enter_context(tc.tile_pool(name="h1p", bufs=2))
        h2_pool = ctx.enter_context(tc.tile_pool(name="h2p", bufs=2))
        ps = ctx.enter_context(tc.tile_pool(name="ps", bufs=1, space="PSUM"))

        # Persistent SBUF tensors
        xaug = singles.tile([128, 32 * BT], TD)   # rhs for z1: x rows + ones rows
        w1b1 = singles.tile([128, 32 * S], TD)    # stationary for z1
        w2sb = singles.tile([S, F * H1], TD)      # W2, s-major
        b2p = singles.tile([2 * H1, 64], F32)     # paired bias columns
        w3p = singles.tile([2 * H1, 64], TD)      # paired W3 columns

        nc.sync.dma_start(out=w2sb, in_=w2t_d)
        nc.sync.dma_start(out=b2p, in_=b2p_d)
        nc.sync.dma_start(out=w3p, in_=w3p_d)
        for i in range(4):
            nc.sync.dma_start(out=w1b1[32 * i : 32 * i + 1, :], in_=w1q_d[i : i + 1, :])
            nc.sync.dma_start(
                out=w1b1[32 * i + 1 : 32 * i + 2, :], in_=b1q_d[i : i + 1, :]
            )
            nc.sync.dma_start(out=xaug[32 * i + 1 : 32 * i + 2, :], in_=ones_d[0:1, :])

        if variant == "pe_only":
            ch1 = singles.tile([128, 2 * BT], TD)
            ch2 = singles.tile([128, 2 * BT], TD)
            nc.vector.memset(ch1, 0.25)
            nc.vector.memset(ch2, 0.25)

        rep_ctx = tc.For_i(0, repeat, 1) if repeat > 1 else None
        if rep_ctx is not None:
            ctx.enter_context(rep_ctx)

        hidx = 0  # global z1-half counter, for ACT/DVE load balancing
        for bt in range(NBT):
            # (re)fill the x rows of xaug for this batch chunk
            for i in range(4):
                nc.sync.dma_start(
                    out=xaug[32 * i : 32 * i + 1, :].rearrange(
                        "p (g j) -> p g j", g=32
                    ),
                    in_=xTg_d[i : i + 1, :, bt * BT : (bt + 1) * BT],
                )

            pout = ps.tile([128, BT], F32, tag="pout")

            if variant == "drains_only":
                # one-time psum fills, then only the drain instruction stream
                pz1a = ps.tile([128, 2 * BT], F32, tag="pz1a")
                pz1b = ps.tile([128, 2 * BT], F32, tag="pz1b")
                pz2 = ps.tile([128, 2 * BT], F32, tag="pz2")
                for i in range(4):
                    dst = pz1a if i < 2 else pz1b
                    nc.tensor.matmul(
                        out=dst[:, (i % 2) * BT : (i % 2 + 1) * BT],
                        lhsT=mv(w1b1[32 * i : 32 * i + 2, 0:S]),
                        rhs=mv(xaug[32 * i : 32 * i + 2, 0:BT]),
                        start=True,
                        stop=True,
                        tile_position=(32 * i, 0),
                    )
                for half in range(2):
                    nc.tensor.matmul(
                        out=pz2[64 * half : 64 * half + 64, 0:BT],
                        lhsT=mv(w2sb[:, 0:H1]),
                        rhs=mv(xaug[0:128, 0:BT]),
                        start=True,
                        stop=True,
                    )
                for g in range(32):
                    h1a = h1_pool.tile([128, 2 * BT], TD, tag="h1a")
                    h1b = h1_pool.tile([128, 2 * BT], TD, tag="h1b")
                    for src, dst in ((pz1a, h1a), (pz1b, h1b)):
                        if hidx % 6 == 5:
                            nc.vector.tensor_scalar_max(dst, src, 0.0)
                        else:
                            nc.scalar.activation(dst, src, Relu)
                        hidx += 1
                    h2 = h2_pool.tile([128, 2 * BT], TD, tag="h2")
                    for pa in range(2):
                        col = 2 * g + pa
                        if hidx % 6 == 5:
                            nc.scalar.activation(
                                h2[:, pa * BT : (pa + 1) * BT],
                                pz2[:, pa * BT : (pa + 1) * BT],
                                Relu,
                                bias=b2p[:, col : col + 1],
                            )
                        else:
                            nc.vector.tensor_scalar(
                                h2[:, pa * BT : (pa + 1) * BT],
                                pz2[:, pa * BT : (pa + 1) * BT],
                                b2p[:, col : col + 1],
                                0.0,
                                mybir.AluOpType.add,
                                mybir.AluOpType.max,
                            )
                # something must reach the output: copy pout anyway
                srow = h2_pool.tile([33, BT], F32, tag="srow")
                nc.vector.memset(srow, 0.0)
                for k in range(2):
                    nc.sync.dma_start(
                        out=out_d[2 * bt + k : 2 * bt + k + 1, :],
                        in_=srow[32 * k : 32 * k + 1, :],
                    )
                continue

            for g in range(32):
                feats = [g, 32 + g, 64 + g, 96 + g]

                # ---- layer 1: z1 = [W1row;b1row].T @ [xrow;ones] , 4-way row-tiled
                pz1a = ps.tile([128, 2 * BT], F32, tag="pz1a")
                pz1b = ps.tile([128, 2 * BT], F32, tag="pz1b")
                for i in range(4):
                    dst = pz1a if i < 2 else pz1b
                    nc.tensor.matmul(
                        out=dst[:, (i % 2) * BT : (i % 2 + 1) * BT],
                        lhsT=mv(w1b1[32 * i : 32 * i + 2, g * S : (g + 1) * S]),
                        rhs=mv(xaug[32 * i : 32 * i + 2, g * BT : (g + 1) * BT]),
                        start=True,
                        stop=True,
                        tile_position=(32 * i, 0),
                    )

                if variant == "pe_only":
                    h1a = h1b = None
                else:
                    # ---- relu drains (split across ACT / DVE)
                    h1a = h1_pool.tile([128, 2 * BT], TD, tag="h1a")
                    h1b = h1_pool.tile([128, 2 * BT], TD, tag="h1b")
                    for src, dst in ((pz1a, h1a), (pz1b, h1b)):
                        if hidx % 6 == 5:
                            nc.vector.tensor_scalar_max(dst, src, 0.0)
                        else:
                            nc.scalar.activation(dst, src, Relu)
                        hidx += 1

                # ---- layer 2: z2 = W2[f].T @ h1[f], feature pairs col-tiled
                pz2 = ps.tile([128, 2 * BT], F32, tag="pz2")
                for pa in range(2):
                    if variant == "pe_only":
                        h1t = ch1
                    else:
                        h1t = h1a if pa == 0 else h1b
                    for half in range(2):
                        f = feats[2 * pa + half]
                        nc.tensor.matmul(
                            out=pz2[
                                64 * half : 64 * half + 64, pa * BT : (pa + 1) * BT
                            ],
                            lhsT=mv(w2sb[:, f * H1 : (f + 1) * H1]),
                            rhs=mv(h1t[:, half * BT : (half + 1) * BT]),
                            start=True,
                            stop=True,
                        )

                if variant == "pe_only":
                    h2 = ch2
                else:
                    # ---- relu(z2 + b2) drains
                    h2 = h2_pool.tile([128, 2 * BT], TD, tag="h2")
                    for pa in range(2):
                        col = 2 * g + pa
                        if hidx % 6 == 5:
                            nc.scalar.activation(
                                h2[:, pa * BT : (pa + 1) * BT],
                                pz2[:, pa * BT : (pa + 1) * BT],
                                Relu,
                                bias=b2p[:, col : col + 1],
                            )
                        else:
                            nc.vector.tensor_scalar(
                                h2[:, pa * BT : (pa + 1) * BT],
                                pz2[:, pa * BT : (pa + 1) * BT],
                                b2p[:, col : col + 1],
                                0.0,
                                mybir.AluOpType.add,
                                mybir.AluOpType.max,
                            )

                # ---- layer 3: out += W3pair.T @ h2pair  (M=1, 2 col positions)
                for pa in range(2):
                    P = 2 * g + pa
                    pos = 32 * (P % 2)
                    nc.tensor.matmul(
                        out=pout[pos : pos + 1, :],
                        lhsT=mv(w3p[:, P : P + 1]),
                        rhs=mv(h2[:, pa * BT : (pa + 1) * BT]),
                        start=(P < 2),
                        stop=(P >= 62),
                        skip_group_check=True,
                    )

            # ---- drain partial rows 0 and 32 to SBUF, then DRAM; host sums them
            srow = h2_pool.tile([33, BT], F32, tag="srow")
            nc.scalar.activation(
                srow, pout[0:33, :], mybir.ActivationFunctionType.Copy
            )
            for k in range(2):
                nc.sync.dma_start(
                    out=out_d[2 * bt + k : 2 * bt + k + 1, :],
                    in_=srow[32 * k : 32 * k + 1, :],
                )

    nc.compile()
    return nc


def _prep_core_inputs(xc, shared, use_f32r=False):
    import ml_dtypes

    dt = np.float32 if use_f32r else ml_dtypes.bfloat16
    m = dict(shared)
    m["xTg"] = np.ascontiguousarray(xc.T).reshape(4, 32, BLOC).astype(dt)
    return m


def _prep_shared(W1, b1, W2, b2, W3, use_f32r=False):
    w1q = np.ascontiguousarray(W1.reshape(4, 32 * S))
    b1q = np.ascontiguousarray(b1.reshape(4, 32 * S))
    w2t = np.ascontiguousarray(W2.transpose(1, 0, 2).reshape(S, F * H1))
    b2p = np.empty((2 * H1, 64), np.float32)
    w3p = np.empty((2 * H1, 64), np.float32)
    W3f = W3.reshape(F, H1)
    for g in range(32):
        b2p[:H1, 2 * g] = b2[g]
        b2p[H1:, 2 * g] = b2[32 + g]
        b2p[:H1, 2 * g + 1] = b2[64 + g]
        b2p[H1:, 2 * g + 1] = b2[96 + g]
        w3p[:H1, 2 * g] = W3f[g]
        w3p[H1:, 2 * g] = W3f[32 + g]
        w3p[:H1, 2 * g + 1] = W3f[64 + g]
        w3p[H1:, 2 * g + 1] = W3f[96 + g]
    import ml_dtypes

    dt = np.float32 if use_f32r else ml_dtypes.bfloat16
    return {
        "w1q": w1q.astype(dt),
        "b1q": b1q.astype(dt),
        "w2t": w2t.astype(dt),
        "b2p": b2p,
        "w3p": w3p.astype(dt),
        "ones": np.ones((1, 32 * BT), dt),
    }


USE_F32R = False


def kernel(x, W1, b1, W2, b2, W3, b3, bias, _trace=False):
    x = np.asarray(x, np.float32)
    W1 = np.asarray(W1, np.float32)
    b1 = np.asarray(b1, np.float32)
    W2 = np.asarray(W2, np.float32)
    b2 = np.asarray(b2, np.float32)
    W3 = np.asarray(W3, np.float32)
    b3 = np.asarray(b3, np.float32)
    bias = np.asarray(bias, np.float32)

    if "nc" not in _CACHE:
        _CACHE["nc"] = _build(use_f32r=USE_F32R)
    nc = _CACHE["nc"]

    shared = _prep_shared(W1, b1, W2, b2, W3, use_f32r=USE_F32R)
    in_maps = [
        _prep_core_inputs(x[c * BLOC : (c + 1) * BLOC], shared, use_f32r=USE_F32R)
        for c in range(NCORES)
    ]

    res = run_bass_kernel_spmd(
        nc, in_maps, core_ids=list(range(NCORES)), trace=_trace
    )
    _CACHE["last_result"] = res

    const = float(b3.sum()) + float(bias.reshape(-1)[0])
    parts = []
    for c in range(NCORES):
        o = res.results[c]["out"]  # [NBT*2, BT]
        parts.append(o.reshape(NBT, 2, BT).sum(axis=1).reshape(BLOC))
    out = np.concatenate(parts) + const
    return out.reshape(B, 1).astype(np.float32)



# revision 4
# speedup vs baseline: 1.7446x; 1.7446x over previous
"""NeuralAdditiveModel TRN2 kernel.

out[b] = sum_f ( relu(relu(x[b,f]*W1[f,:]+b1[f,:]) @ W2[f] + b2[f]) @ W3[f] + b3[f] ) + bias

Sharding: data-parallel over batch, 8 cores x 1024 rows. No collectives.

Per-core dataflow: 64 groups, each 4 features x 512-batch chunk, software
pipelined. PE runs at 4 phases of ~512 cycles per group (this part's PE is
clock-limited, so phase count is what matters):
  z1 : four K=2 matmuls row-tiled on all 4 strips (one phase) -> pz1_s0/s1
  z2 : two col-tiled M=64 matmuls per slot (two phases)       -> pz2_s0/s1
  z3 : two M=1 matmuls, col strips 0/1, concurrent (one phase) -> pout rows 0/32
Drains (relu / relu+bias) are split ACT vs DVE, ~balanced.

Group G=(bt,g) covers feats {g, g+64} (slot s0) and {g+32, g+96} (slot s1);
issue order: z1(G+1), z2(G), h1(G+1), z3(G-1), h2(G). PSUM = exactly 8 banks
(pz1 4, pz2 2, pout 2).
"""

import sys
from contextlib import ExitStack

import numpy as np

sys.path.insert(0, "/opt/trn_rl_repo")

import concourse.bass as bass  # noqa: E402
import concourse.tile as tile  # noqa: E402
from concourse import bacc, mybir  # noqa: E402
from concourse.bass_utils import run_bass_kernel_spmd  # noqa: E402

B, F, S, H1 = 8192, 128, 128, 64
NCORES = 8
BLOC = B // NCORES   # 1024 rows per core
BT = 512             # batch chunk (PSUM bank width in fp32)
NBT = BLOC // BT     # 2
NG = 32              # feature groups per chunk
NGRP = NBT * NG      # 64 pipeline groups
F32 = mybir.dt.float32
BF16 = mybir.dt.bfloat16

_CACHE = {}


def _build():
    nc = bacc.Bacc(
        "TRN2",
        target_bir_lowering=False,
        debug=False,
        enable_asserts=False,
        num_devices=NCORES,
    )

    xg_d = nc.dram_tensor("xg", [4, 32 * BLOC], BF16, kind="ExternalInput").ap()
    ones_d = nc.dram_tensor("ones", [1, 32 * BLOC], BF16, kind="ExternalInput").ap()
    w1q_d = nc.dram_tensor("w1q", [4, 32 * S], BF16, kind="ExternalInput").ap()
    b1q_d = nc.dram_tensor("b1q", [4, 32 * S], BF16, kind="ExternalInput").ap()
    w2t_d = nc.dram_tensor("w2t", [S, F * H1], BF16, kind="ExternalInput").ap()
    b2p_d = nc.dram_tensor("b2p", [2 * H1, F // 2], F32, kind="ExternalInput").ap()
    w3p_d = nc.dram_tensor("w3p", [2 * H1, F // 2], BF16, kind="ExternalInput").ap()
    out_d = nc.dram_tensor("out", [NBT * 2, BT], F32, kind="ExternalOutput").ap()

    Relu = mybir.ActivationFunctionType.Relu
    Copy = mybir.ActivationFunctionType.Copy

    with tile.TileContext(nc) as tc, ExitStack() as ctx:
        singles = ctx.enter_context(tc.tile_pool(name="singles", bufs=1))
        h1_pool = ctx.enter_context(tc.tile_pool(name="h1p", bufs=2))
        h2_pool = ctx.enter_context(tc.tile_pool(name="h2p", bufs=2))
        ps = ctx.enter_context(tc.tile_pool(name="ps", bufs=1, space="PSUM"))

        # Persistent SBUF tensors
        xaug = singles.tile([128, 32 * BLOC], BF16)  # x rows (32i) + ones (32i+1)
        w1b1 = singles.tile([128, 32 * S], BF16)     # W1 rows (32i) + b1 (32i+1)
        w2sb = singles.tile([S, F * H1], BF16)       # W2, s-major
        b2p = singles.tile([2 * H1, F // 2], F32)    # paired bias columns
        w3p = singles.tile([2 * H1, F // 2], BF16)   # paired W3 columns

        for i in range(4):
            nc.sync.dma_start(out=w1b1[32 * i : 32 * i + 1, :], in_=w1q_d[i : i + 1, :])
            nc.sync.dma_start(
                out=w1b1[32 * i + 1 : 32 * i + 2, :], in_=b1q_d[i : i + 1, :]
            )
            nc.sync.dma_start(out=xaug[32 * i : 32 * i + 1, :], in_=xg_d[i : i + 1, :])
            nc.sync.dma_start(out=xaug[32 * i + 1 : 32 * i + 2, :], in_=ones_d[0:1, :])
        FQ = F // 4 * H1  # w2t column quarter
        for c in range(4):
            nc.sync.dma_start(
                out=w2sb[:, c * FQ : (c + 1) * FQ], in_=w2t_d[:, c * FQ : (c + 1) * FQ]
            )
        nc.sync.dma_start(out=b2p, in_=b2p_d)
        nc.sync.dma_start(out=w3p, in_=w3p_d)

        def grp(G):  # group -> (bt, g)
            return G // NG, G % NG

        def z1(G, pza, pzb):
            bt, g = grp(G)
            # strips q0,q32,q64,q96 <-> feats g, g+32, g+64, g+96; 4 banks
            for i, pz, half in ((0, pza, 0), (1, pzb, 0), (2, pza, 1), (3, pzb, 1)):
                r = 32 * i
                nc.tensor.matmul(
                    out=pz[:, half * BT : (half + 1) * BT],
                    lhsT=w1b1[r : r + 2, g * S : (g + 1) * S],
                    rhs=xaug[r : r + 2, g * BLOC + bt * BT : g * BLOC + (bt + 1) * BT],
                    start=True,
                    stop=True,
                    tile_position=(r, 0),
                )

        def z2(G, sub, h1sb, pz2):
            _, g = grp(G)
            j = g + 32 * sub
            for half, f in enumerate((j, j + 64)):
                nc.tensor.matmul(
                    out=pz2[64 * half : 64 * half + 64, :],
                    lhsT=w2sb[:, f * H1 : (f + 1) * H1],
                    rhs=h1sb[:, half * BT : (half + 1) * BT],
                    start=True,
                    stop=True,
                )

        def h1drain(G, sub, pz, h1sb):
            if sub == 0:
                nc.vector.tensor_scalar_max(h1sb, pz, 0.0)
            else:
                nc.scalar.activation(h1sb, pz, Relu)

        def h2drain(G, sub, pz2, h2sb):
            _, g = grp(G)
            j = g + 32 * sub
            if sub == 0:
                nc.scalar.activation(h2sb, pz2, Relu, bias=b2p[:, j : j + 1])
            else:
                nc.vector.tensor_scalar(
                    h2sb,
                    pz2,
                    b2p[:, j : j + 1],
                    0.0,
                    mybir.AluOpType.add,
                    mybir.AluOpType.max,
                )

        def z3(G, sub, h2sb, pout):
            _, g = grp(G)
            j = g + 32 * sub
            nc.tensor.matmul(
                out=pout[32 * sub : 32 * sub + 1, :],
                lhsT=w3p[:, j : j + 1],
                rhs=h2sb,
                start=(g == 0),
                stop=(g == NG - 1),
                skip_group_check=True,
            )

        pz1a_t = [None] * NGRP  # slot s0 z1 psum
        pz1b_t = [None] * NGRP  # slot s1 z1 psum
        h1a_t = [None] * NGRP
        h1b_t = [None] * NGRP
        pz2a_t = [None] * NGRP
        pz2b_t = [None] * NGRP
        h2a_t = [None] * NGRP
        h2b_t = [None] * NGRP
        pout_t = [None] * NBT

        def alloc_z1(G):
            pz1a_t[G] = ps.tile([128, 2 * BT], F32, tag="pz1a", name="pz1a")
            pz1b_t[G] = ps.tile([128, 2 * BT], F32, tag="pz1b", name="pz1b")

        def alloc_h1(G):
            h1a_t[G] = h1_pool.tile([128, 2 * BT], BF16, tag="h1a", name="h1a")
            h1b_t[G] = h1_pool.tile([128, 2 * BT], BF16, tag="h1b", name="h1b")

        alloc_z1(0)
        z1(0, pz1a_t[0], pz1b_t[0])
        alloc_h1(0)
        h1drain(0, 0, pz1a_t[0], h1a_t[0])
        h1drain(0, 1, pz1b_t[0], h1b_t[0])

        for G in range(NGRP):
            if G + 1 < NGRP:
                alloc_z1(G + 1)
                z1(G + 1, pz1a_t[G + 1], pz1b_t[G + 1])
            pz2a_t[G] = ps.tile([128, BT], F32, tag="pz2a", name="pz2a")
            z2(G, 0, h1a_t[G], pz2a_t[G])
            pz2b_t[G] = ps.tile([128, BT], F32, tag="pz2b", name="pz2b")
            z2(G, 1, h1b_t[G], pz2b_t[G])
            if G + 1 < NGRP:
                alloc_h1(G + 1)
                h1drain(G + 1, 0, pz1a_t[G + 1], h1a_t[G + 1])
                h1drain(G + 1, 1, pz1b_t[G + 1], h1b_t[G + 1])
            if G >= 1:
                bt, g = grp(G - 1)
                if g == 0:
                    pout_t[bt] = ps.tile([128, BT], F32, tag="pout", name="pout", bufs=2)
                z3(G - 1, 0, h2a_t[G - 1], pout_t[bt])
                z3(G - 1, 1, h2b_t[G - 1], pout_t[bt])
                if g == NG - 1:
                    srow = h2_pool.tile([33, BT], F32, tag="srow", name="srow")
                    nc.scalar.activation(srow, pout_t[bt][0:33, :], Copy)
                    for k in range(2):
                        nc.sync.dma_start(
                            out=out_d[2 * bt + k : 2 * bt + k + 1, :],
                            in_=srow[32 * k : 32 * k + 1, :],
                        )
            h2a_t[G] = h2_pool.tile([128, BT], BF16, tag="h2a", name="h2a")
            h2drain(G, 0, pz2a_t[G], h2a_t[G])
            h2b_t[G] = h2_pool.tile([128, BT], BF16, tag="h2b", name="h2b")
            h2drain(G, 1, pz2b_t[G], h2b_t[G])

        # epilogue
        G = NGRP
        bt, g = grp(G - 1)
        z3(G - 1, 0, h2a_t[G - 1], pout_t[bt])
        z3(G - 1, 1, h2b_t[G - 1], pout_t[bt])
        srow = h2_pool.tile([33, BT], F32, tag="srow", name="srow")
        nc.scalar.activation(srow, pout_t[bt][0:33, :], Copy)
        for k in range(2):
            nc.sync.dma_start(
                out=out_d[2 * bt + k : 2 * bt + k + 1, :],
                in_=srow[32 * k : 32 * k + 1, :],
            )

    nc.compile()
    return nc


def _prep_shared(W1, b1, W2, b2, W3):
    import ml_dtypes

    bf = ml_dtypes.bfloat16
    w1q = np.ascontiguousarray(W1.reshape(4, 32 * S)).astype(bf)
    b1q = np.ascontiguousarray(b1.reshape(4, 32 * S)).astype(bf)
    w2t = np.ascontiguousarray(W2.transpose(1, 0, 2).reshape(S, F * H1)).astype(bf)
    b2p = np.empty((2 * H1, F // 2), np.float32)
    w3p = np.empty((2 * H1, F // 2), np.float32)
    W3f = W3.reshape(F, H1)
    for j in range(F // 2):
        b2p[:H1, j] = b2[j]
        b2p[H1:, j] = b2[j + 64]
        w3p[:H1, j] = W3f[j]
        w3p[H1:, j] = W3f[j + 64]
    return {
        "w1q": w1q,
        "b1q": b1q,
        "w2t": w2t,
        "b2p": b2p,
        "w3p": w3p.astype(bf),
        "ones": np.ones((1, 32 * BLOC), bf),
    }


def _prep_core_inputs(xc, shared):
    import ml_dtypes

    m = dict(shared)
    # xg[i, g*BLOC + b] = x[b, 32i+g]
    m["xg"] = (
        np.ascontiguousarray(xc.T.reshape(4, 32 * BLOC)).astype(ml_dtypes.bfloat16)
    )
    return m


def kernel(x, W1, b1, W2, b2, W3, b3, bias, _trace=False):
    x = np.asarray(x, np.float32)
    W1 = np.asarray(W1, np.float32)
    b1 = np.asarray(b1, np.float32)
    W2 = np.asarray(W2, np.float32)
    b2 = np.asarray(b2, np.float32)
    W3 = np.asarray(W3, np.float32)
    b3 = np.asarray(b3, np.float32)
    bias = np.asarray(bias, np.float32)

    if "nc" not in _CACHE:
        _CACHE["nc"] = _build()
    nc = _CACHE["nc"]

    shared = _prep_shared(W1, b1, W2, b2, W3)
    in_maps = [
        _prep_core_inputs(x[c * BLOC : (c + 1) * BLOC], shared) for c in range(NCORES)
    ]

    res = run_bass_kernel_spmd(nc, in_maps, core_ids=list(range(NCORES)), trace=_trace)
    _CACHE["last_result"] = res

    const = float(b3.sum()) + float(bias.reshape(-1)[0])
    parts = []
    for c in range(NCORES):
        o = res.results[c]["out"]  # [NBT*2, BT]: rows 2bt,2bt+1 = pout rows 0,32
        parts.append(o.reshape(NBT, 2, BT).sum(axis=1).reshape(BLOC))
    out = np.concatenate(parts) + const
    return out.reshape(B, 1).astype(np.float32)


# revision 6
# speedup vs baseline: 1.8566x; 1.0642x over previous
"""NeuralAdditiveModel TRN2 kernel.

out[b] = sum_f ( relu(relu(x[b,f]*W1[f,:]+b1[f,:]) @ W2[f] + b2[f]) @ W3[f] + b3[f] ) + bias

Sharding: data-parallel over batch, 8 cores x 1024 rows. No collectives.

Per-core dataflow: 64 groups, each 4 features x 512-batch chunk, software
pipelined. PE runs at 4 phases of ~512 cycles per group (this part's PE is
clock-limited, so phase count is what matters):
  z1 : four K=2 matmuls row-tiled on all 4 strips (one phase) -> pz1_s0/s1
  z2 : two col-tiled M=64 matmuls per slot (two phases)       -> pz2_s0/s1
  z3 : two M=1 matmuls, col strips 0/1, concurrent (one phase) -> pout rows 0/32
Drains (relu / relu+bias) are split ACT vs DVE, ~balanced.

Group G=(bt,g) covers feats {g, g+64} (slot s0) and {g+32, g+96} (slot s1);
issue order: z1(G+1), z2(G), h1(G+1), z3(G-1), h2(G). PSUM = exactly 8 banks
(pz1 4, pz2 2, pout 2).
"""

import sys
from contextlib import ExitStack

import numpy as np

sys.path.insert(0, "/opt/trn_rl_repo")

import concourse.bass as bass  # noqa: E402
import concourse.tile as tile  # noqa: E402
from concourse import bacc, mybir  # noqa: E402
from concourse.bass_utils import run_bass_kernel_spmd  # noqa: E402

B, F, S, H1 = 8192, 128, 128, 64
NCORES = 8
BLOC = B // NCORES   # 1024 rows per core
BT = 512             # batch chunk (PSUM bank width in fp32)
NBT = BLOC // BT     # 2
NG = 32              # feature groups per chunk
NGRP = NBT * NG      # 64 pipeline groups
F32 = mybir.dt.float32
BF16 = mybir.dt.bfloat16

_CACHE = {}


def _build():
    nc = bacc.Bacc(
        "TRN2",
        target_bir_lowering=False,
        debug=False,
        enable_asserts=False,
        num_devices=NCORES,
    )

    xg_d = nc.dram_tensor("xg", [4, 32 * BLOC], BF16, kind="ExternalInput").ap()
    ones_d = nc.dram_tensor("ones", [1, 32 * BLOC], BF16, kind="ExternalInput").ap()
    w1q_d = nc.dram_tensor("w1q", [4, 32 * S], BF16, kind="ExternalInput").ap()
    b1q_d = nc.dram_tensor("b1q", [4, 32 * S], BF16, kind="ExternalInput").ap()
    w2t_d = nc.dram_tensor("w2t", [S, F * H1], BF16, kind="ExternalInput").ap()
    b2p_d = nc.dram_tensor("b2p", [2 * H1, F // 2], F32, kind="ExternalInput").ap()
    w3p_d = nc.dram_tensor("w3p", [2 * H1, F // 2], BF16, kind="ExternalInput").ap()
    out_d = nc.dram_tensor("out", [NBT * 2, BT], F32, kind="ExternalOutput").ap()

    Relu = mybir.ActivationFunctionType.Relu
    Copy = mybir.ActivationFunctionType.Copy

    with tile.TileContext(nc) as tc, ExitStack() as ctx:
        singles = ctx.enter_context(tc.tile_pool(name="singles", bufs=1))
        h1_pool = ctx.enter_context(tc.tile_pool(name="h1p", bufs=2))
        h2_pool = ctx.enter_context(tc.tile_pool(name="h2p", bufs=2))
        ps = ctx.enter_context(tc.tile_pool(name="ps", bufs=1, space="PSUM"))

        # Persistent SBUF tensors
        xaug = singles.tile([128, 32 * BLOC], BF16)  # x rows (32i) + ones (32i+1)
        w1b1 = singles.tile([128, 32 * S], BF16)     # W1 rows (32i) + b1 (32i+1)
        w2sb = singles.tile([S, F * H1], BF16)       # W2, s-major
        b2p = singles.tile([2 * H1, F // 2], F32)    # paired bias columns
        w3p = singles.tile([2 * H1, F // 2], BF16)   # paired W3 columns

        # Setup DMAs spread over the 5 engine queues so the single-partition
        # row transfers and the big w2 pull run in parallel (one queue
        # serializes them into a ~35us ramp).
        w1b1_g = w1b1.rearrange("(i q) c -> i q c", q=32)
        nc.gpsimd.dma_start(out=w1b1_g[:, 0, :], in_=w1q_d)
        nc.gpsimd.dma_start(out=w1b1_g[:, 1, :], in_=b1q_d)
        nc.sync.dma_start(out=xaug[1:2, :], in_=ones_d[0:1, :])
        nc.scalar.dma_start(out=xaug[33:34, :], in_=ones_d[0:1, :])
        nc.gpsimd.dma_start(out=xaug[65:66, :], in_=ones_d[0:1, :])
        nc.gpsimd.dma_start(out=xaug[97:98, :], in_=ones_d[0:1, :])
        xq = (nc.sync, nc.scalar, nc.sync, nc.scalar)
        for i in range(4):
            xq[i].dma_start(out=xaug[32 * i : 32 * i + 1, :], in_=xg_d[i : i + 1, :])
        FQ = F // 4 * H1  # w2t column quarter
        w2q = (nc.sync, nc.sync, nc.scalar, nc.scalar)
        for c in (0, 2, 1, 3):
            w2q[c].dma_start(
                out=w2sb[:, c * FQ : (c + 1) * FQ], in_=w2t_d[:, c * FQ : (c + 1) * FQ]
            )
        nc.sync.dma_start(out=b2p, in_=b2p_d)
        nc.sync.dma_start(out=w3p, in_=w3p_d)

        def grp(G):  # group -> (bt, g)
            return G // NG, G % NG

        def z1(G, pza, pzb):
            bt, g = grp(G)
            # strips q0,q32,q64,q96 <-> feats g, g+32, g+64, g+96; 4 banks
            for i, pz, half in ((0, pza, 0), (1, pzb, 0), (2, pza, 1), (3, pzb, 1)):
                r = 32 * i
                nc.tensor.matmul(
                    out=pz[:, half * BT : (half + 1) * BT],
                    lhsT=w1b1[r : r + 2, g * S : (g + 1) * S],
                    rhs=xaug[r : r + 2, g * BLOC + bt * BT : g * BLOC + (bt + 1) * BT],
                    start=True,
                    stop=True,
                    tile_position=(r, 0),
                )

        def z2(G, sub, h1sb, pz2):
            _, g = grp(G)
            j = g + 32 * sub
            for half, f in enumerate((j, j + 64)):
                nc.tensor.matmul(
                    out=pz2[64 * half : 64 * half + 64, :],
                    lhsT=w2sb[:, f * H1 : (f + 1) * H1],
                    rhs=h1sb[:, half * BT : (half + 1) * BT],
                    start=True,
                    stop=True,
                )

        def h1drain(G, sub, pz, h1sb):
            if sub == 0:
                nc.vector.tensor_scalar_max(h1sb, pz, 0.0)
            else:
                nc.scalar.activation(h1sb, pz, Relu)

        def h2drain(G, sub, pz2, h2sb):
            _, g = grp(G)
            j = g + 32 * sub
            if sub == 0:
                nc.scalar.activation(h2sb, pz2, Relu, bias=b2p[:, j : j + 1])
            else:
                nc.vector.tensor_scalar(
                    h2sb,
                    pz2,
                    b2p[:, j : j + 1],
                    0.0,
                    mybir.AluOpType.add,
                    mybir.AluOpType.max,
                )

        def z3(G, sub, h2sb, pout):
            _, g = grp(G)
            j = g + 32 * sub
            nc.tensor.matmul(
                out=pout[32 * sub : 32 * sub + 1, :],
                lhsT=w3p[:, j : j + 1],
                rhs=h2sb,
                start=(g == 0),
                stop=(g == NG - 1),
                skip_group_check=True,
            )

        pz1a_t = [None] * NGRP  # slot s0 z1 psum
        pz1b_t = [None] * NGRP  # slot s1 z1 psum
        h1a_t = [None] * NGRP
        h1b_t = [None] * NGRP
        pz2a_t = [None] * NGRP
        pz2b_t = [None] * NGRP
        h2a_t = [None] * NGRP
        h2b_t = [None] * NGRP
        pout_t = [None] * NBT

        def alloc_z1(G):
            pz1a_t[G] = ps.tile([128, 2 * BT], F32, tag="pz1a", name="pz1a")
            pz1b_t[G] = ps.tile([128, 2 * BT], F32, tag="pz1b", name="pz1b")

        def alloc_h1(G):
            h1a_t[G] = h1_pool.tile([128, 2 * BT], BF16, tag="h1a", name="h1a")
            h1b_t[G] = h1_pool.tile([128, 2 * BT], BF16, tag="h1b", name="h1b")

        alloc_z1(0)
        z1(0, pz1a_t[0], pz1b_t[0])
        alloc_h1(0)
        h1drain(0, 0, pz1a_t[0], h1a_t[0])
        h1drain(0, 1, pz1b_t[0], h1b_t[0])

        for G in range(NGRP):
            if G + 1 < NGRP:
                alloc_z1(G + 1)
                z1(G + 1, pz1a_t[G + 1], pz1b_t[G + 1])
            pz2a_t[G] = ps.tile([128, BT], F32, tag="pz2a", name="pz2a")
            z2(G, 0, h1a_t[G], pz2a_t[G])
            pz2b_t[G] = ps.tile([128, BT], F32, tag="pz2b", name="pz2b")
            z2(G, 1, h1b_t[G], pz2b_t[G])
            if G + 1 < NGRP:
                alloc_h1(G + 1)
                h1drain(G + 1, 0, pz1a_t[G + 1], h1a_t[G + 1])
                h1drain(G + 1, 1, pz1b_t[G + 1], h1b_t[G + 1])
            if G >= 1:
                bt, g = grp(G - 1)
                if g == 0:
                    pout_t[bt] = ps.tile([128, BT], F32, tag="pout", name="pout", bufs=2)
                z3(G - 1, 0, h2a_t[G - 1], pout_t[bt])
                z3(G - 1, 1, h2b_t[G - 1], pout_t[bt])
                if g == NG - 1:
                    srow = h2_pool.tile([33, BT], F32, tag="srow", name="srow")
                    nc.scalar.activation(srow, pout_t[bt][0:33, :], Copy)
                    for k in range(2):
                        nc.sync.dma_start(
                            out=out_d[2 * bt + k : 2 * bt + k + 1, :],
                            in_=srow[32 * k : 32 * k + 1, :],
                        )
            h2a_t[G] = h2_pool.tile([128, BT], BF16, tag="h2a", name="h2a")
            h2drain(G, 0, pz2a_t[G], h2a_t[G])
            h2b_t[G] = h2_pool.tile([128, BT], BF16, tag="h2b", name="h2b")
            h2drain(G, 1, pz2b_t[G], h2b_t[G])

        # epilogue
        G = NGRP
        bt, g = grp(G - 1)
        z3(G - 1, 0, h2a_t[G - 1], pout_t[bt])
        z3(G - 1, 1, h2b_t[G - 1], pout_t[bt])
        srow = h2_pool.tile([33, BT], F32, tag="srow", name="srow")
        nc.scalar.activation(srow, pout_t[bt][0:33, :], Copy)
        for k in range(2):
            nc.sync.dma_start(
                out=out_d[2 * bt + k : 2 * bt + k + 1, :],
                in_=srow[32 * k : 32 * k + 1, :],
            )

    nc.compile()
    return nc


def _prep_shared(W1, b1, W2, b2, W3):
    import ml_dtypes

    bf = ml_dtypes.bfloat16
    w1q = np.ascontiguousarray(W1.reshape(4, 32 * S)).astype(bf)
    b1q = np.ascontiguousarray(b1.reshape(4, 32 * S)).astype(bf)
    w2t = np.ascontiguousarray(W2.transpose(1, 0, 2).reshape(S, F * H1)).astype(bf)
    b2p = np.empty((2 * H1, F // 2), np.float32)
    w3p = np.empty((2 * H1, F // 2), np.float32)
    W3f = W3.reshape(F, H1)
    for j in range(F // 2):
        b2p[:H1, j] = b2[j]
        b2p[H1:, j] = b2[j + 64]
        w3p[:H1, j] = W3f[j]
        w3p[H1:, j] = W3f[j + 64]
    return {
        "w1q": w1q,
        "b1q": b1q,
        "w2t": w2t,
        "b2p": b2p,
        "w3p": w3p.astype(bf),
        "ones": np.ones((1, 32 * BLOC), bf),
    }


def _prep_core_inputs(xc, shared):
    import ml_dtypes

    m = dict(shared)
    # xg[i, g*BLOC + b] = x[b, 32i+g]
    m["xg"] = (
        np.ascontiguousarray(xc.T.reshape(4, 32 * BLOC)).astype(ml_dtypes.bfloat16)
    )
    return m


def kernel(x, W1, b1, W2, b2, W3, b3, bias, _trace=False):
    x = np.asarray(x, np.float32)
    W1 = np.asarray(W1, np.float32)
    b1 = np.asarray(b1, np.float32)
    W2 = np.asarray(W2, np.float32)
    b2 = np.asarray(b2, np.float32)
    W3 = np.asarray(W3, np.float32)
    b3 = np.asarray(b3, np.float32)
    bias = np.asarray(bias, np.float32)

    if "nc" not in _CACHE:
        _CACHE["nc"] = _build()
    nc = _CACHE["nc"]

    shared = _prep_shared(W1, b1, W2, b2, W3)
    in_maps = [
        _prep_core_inputs(x[c * BLOC : (c + 1) * BLOC], shared) for c in range(NCORES)
    ]

    res = run_bass_kernel_spmd(nc, in_maps, core_ids=list(range(NCORES)), trace=_trace)
    _CACHE["last_result"] = res

    const = float(b3.sum()) + float(bias.reshape(-1)[0])
    parts = []
    for c in range(NCORES):
        o = res.results[c]["out"]  # [NBT*2, BT]: rows 2bt,2bt+1 = pout rows 0,32
        parts.append(o.reshape(NBT, 2, BT).sum(axis=1).reshape(BLOC))
    out = np.concatenate(parts) + const
    return out.reshape(B, 1).astype(np.float32)
